# revision 36
# baseline (speedup 1.0000x reference)
"""Differential attention (B=2, T=2048, D=2048, H=16, HD=128) on 8 TRN2 cores.

Sharding: core c -> (batch b = c // 4, head-group g = c % 4); each core runs
batch b with 4 heads (4g..4g+3). Out-projection partials are summed on host
(outputs stored bf16; host accumulates in f32).

Per-core schedule (single SPMD Bass program), changes over the 468us
baseline (measured 447-452us on the same harness; baseline re-measured
~558us in this session):
  - attn transposes moved OFF the PE: dn rows are transposed via the DMA
    XBAR (dma_start(transpose=True), 16x128 tiles; ONE instr per q-tile
    batching up to 16 128x128 blocks through a 3D out AP into a single
    [128, 8192] aTs tile per phase). PE sheds ~70k columns + 544 matmul
    overheads; the psum aT ring and its evac casts disappear. 64 XBARs
    total keeps the SP queue (which also carries x/out DMAs) unclogged.
    Removing the interleaved transposes also fixes the projection-chain
    cadence: back-to-back 512-col matmuls now issue at ~216ns (2.4GHz)
    vs 259ns in the baseline.
  - score pairs (q1k1 / q2k2, K=64 each) rely on bass's auto tile_position
    (row groups 0/64) and execute CONCURRENTLY in the PE array (verified
    4ns start stagger on HW) - scores cost N cols per pair, not 2N.
  - normalization restructured since the diag-matmul trick is gone:
    dn = e1*(1/l1) + (-lam/l2)*e2 built as g = e1*r1 (DVE tensor_scalar,
    2 elem/cycle) and dn = stt(e2, ccr, g, mult, add) on DVE with bf16
    ccr; lam uploaded NEGATED from host. No diag tiles, no idnn.
    (GpSimd was tried for g and is ~15x too slow - 6.4us per [128,512].)
  - softmax runs on 1024-wide superchunks (2-bank psum tiles, one
    exp+accum per half): halves ACTIVATE / ACTIVATION_READ_ACCUMULATOR /
    stt / XBAR instruction counts vs 512 chunks.
  - weights land in per-4d-group SBUF tiles (wq/wk/wv: 4x[128,2048], wo:
    2x[128,4096]) so first matmuls gate on 512KB not 2MB; DMA queues:
    SP=x+wv+out+xbar, Act=wq+wk+wo.
  - psum: scps 2x[128,1024] + prjps 3x[128,512] + av accumulator; the
    3-deep projection ring plus a 3-deep oev pool keeps the out-projection
    weave from stalling on out-DMA backlog.

All matmuls bf16; accumulation fp32 in PSUM, softmax statistics fp32.
"""

from contextlib import ExitStack

import ml_dtypes
import numpy as np

B, T, D = 2, 2048, 2048
H, HD = 16, 128
HHD = HD // 2
HL = 4  # heads per core
NCORES = 8
SCALE = 1.0 / float(np.sqrt(np.float32(HHD)))

TB = 512  # t-superblock (q-block rows, AV free dim)
NTB = T // TB  # 4
DC = 128  # contraction chunk (partition dim)
NDC = D // DC  # 16
NQT = TB // 128  # q-tiles (128 rows) per superblock
CH = 512  # softmax chunk width (1 PSUM bank)

_CACHE = {}


def _build():
    import concourse.mybir as mybir
    from concourse.bacc import Bacc
    from concourse.tile import TileContext

    f32 = mybir.dt.float32
    bf16 = mybir.dt.bfloat16
    Alu = mybir.AluOpType
    Act = mybir.ActivationFunctionType
    X = mybir.AxisListType.X

    nc = Bacc("TRN2", num_devices=NCORES)
    # host-rearranged inputs: xr[p, tb*8192 + d*512 + j] = x[b][tb*512+j, d*128+p]
    xr = nc.declare_dram_parameter("xr", [128, NTB * NDC * TB], bf16, isOutput=False)
    # wq/wk/wv: w_r[p, d*512 + c] = W[d*128+p, g*512 + c]
    wq = nc.declare_dram_parameter("wq", [128, NDC * HL * HD], bf16, isOutput=False)
    wk = nc.declare_dram_parameter("wk", [128, NDC * HL * HD], bf16, isOutput=False)
    wv = nc.declare_dram_parameter("wv", [128, NDC * HL * HD], bf16, isOutput=False)
    # won_r[p, h*2048 + n] = Wo[g*512 + h*128 + p, n]
    won = nc.declare_dram_parameter("won", [128, HL * D], bf16, isOutput=False)
    lamn = nc.declare_dram_parameter("lamn", [128, HL], f32, isOutput=False)  # -sigmoid
    msk = nc.declare_dram_parameter("msk", [128, 128], f32, isOutput=False)
    out = nc.declare_dram_parameter("out", [T, D], bf16, isOutput=True)

    WG = 4 * 512  # 4-d-group weight tile cols

    with TileContext(nc) as tc, ExitStack() as top:
        # ---- persistent SBUF ----
        const = top.enter_context(tc.tile_pool(name="const", bufs=1))
        lamn_sb = const.tile([128, HL], f32, tag="lam", name="lam")
        msk_sb = const.tile([128, 128], f32, tag="msk", name="msk")

        wpool = top.enter_context(tc.tile_pool(name="wpool", bufs=1))
        wq_t = [wpool.tile([128, WG], bf16, tag=f"wq{g}", name=f"wq{g}") for g in range(4)]
        wk_t = [wpool.tile([128, WG], bf16, tag=f"wk{g}", name=f"wk{g}") for g in range(4)]
        wv_t = [wpool.tile([128, WG], bf16, tag=f"wv{g}", name=f"wv{g}") for g in range(4)]
        wo_t = [wpool.tile([128, 2 * D], bf16, tag=f"wo{k}", name=f"wo{k}") for k in range(2)]

        resid = top.enter_context(tc.tile_pool(name="resid", bufs=1))
        kt_sb = [resid.tile([128, T], bf16, tag=f"kt{h}", name=f"kt{h}") for h in range(HL)]
        v_sb = [resid.tile([128, HL * HD], bf16, tag=f"v{s}", name=f"v{s}") for s in range(T // 128)]
        ot_sb = [resid.tile([128, T], bf16, tag=f"ot{h}", name=f"ot{h}") for h in range(HL)]
        # q tiles per (h, tb): alive ~1.5 superblocks -> 8-buf rotation
        qpool = top.enter_context(tc.tile_pool(name="qpool", bufs=7))
        qt_sb = {}

        xpool = top.enter_context(tc.tile_pool(name="xpool", bufs=2))
        x_sb = {}

        def wslice(tiles, d, h0c, h1c):
            """weight cols [d*512 + h0c, d*512 + h1c) from the 4-d-group tiles."""
            return tiles[d // 4][:, (d % 4) * 512 + h0c : (d % 4) * 512 + h1c]

        # ---- input DMAs ----
        def emit_xdma(tb, widths=(1024,) * 8):
            xt = xpool.tile([128, NDC * TB], bf16, tag="x", name=f"x{tb}")
            off = 0
            for cw in widths:
                nc.sync.dma_start(
                    out=xt[:, off : off + cw],
                    in_=xr[:, tb * NDC * TB + off : tb * NDC * TB + off + cw],
                )
                off += cw
            x_sb[tb] = xt

        emit_xdma(0, widths=(512, 512) + (1024,) * 7)
        for g in range(4):
            if g == 0:
                nc.scalar.dma_start(out=wq_t[0][:, 0:512], in_=wq[:, 0:512])
                nc.scalar.dma_start(out=wq_t[0][:, 512:WG], in_=wq[:, 512:WG])
            else:
                nc.scalar.dma_start(out=wq_t[g][:], in_=wq[:, g * WG : (g + 1) * WG])
        for g in range(4):
            nc.scalar.dma_start(out=wk_t[g][:], in_=wk[:, g * WG : (g + 1) * WG])
        nc.sync.dma_start(out=lamn_sb[:], in_=lamn[:])
        nc.sync.dma_start(out=msk_sb[:], in_=msk[:])
        for g in range(4):
            nc.sync.dma_start(out=wv_t[g][:], in_=wv[:, g * WG : (g + 1) * WG])
        for k in range(2):
            nc.scalar.dma_start(out=wo_t[k][:], in_=won[:, k * 2 * D : (k + 1) * 2 * D])

        # ---- prologue: Q(tb0), K(tb0) on a wide psum pool ----
        with ExitStack() as ph1:
            pps = ph1.enter_context(tc.tile_pool(name="pps", bufs=8, space="PSUM"))
            xt0 = x_sb[0]
            for w_t, store in ((wq_t, "q"), (wk_t, "k")):
                pss = [pps.tile([128, TB], f32, tag="ps", name=f"p{h}", bufs=4) for h in range(HL)]
                for d in range(NDC):
                    for h in range(HL):
                        nc.tensor.matmul(
                            pss[h][:],
                            lhsT=wslice(w_t, d, h * HD, (h + 1) * HD),
                            rhs=xt0[:, d * TB : (d + 1) * TB],
                            start=(d == 0),
                            stop=(d == NDC - 1),
                        )
                for h in range(HL):
                    if store == "q":
                        qt = qpool.tile([128, TB], bf16, tag="qt", name=f"qt0_{h}")
                        nc.vector.tensor_copy(qt[:], pss[h][:])
                        qt_sb[(0, h)] = qt
                    else:
                        nc.vector.tensor_copy(kt_sb[h][:, 0:TB], pss[h][:])

        # ---------------- attention + late projections ----------------
        from collections import deque

        with ExitStack() as ph2:
            scps = ph2.enter_context(tc.tile_pool(name="scps", bufs=2, space="PSUM"))
            prjps = ph2.enter_context(tc.tile_pool(name="prjps", bufs=3, space="PSUM"))
            accps = ph2.enter_context(tc.tile_pool(name="accps", bufs=1, space="PSUM"))
            epool = ph2.enter_context(tc.tile_pool(name="epool", bufs=5))
            dpool = ph2.enter_context(tc.tile_pool(name="dpool", bufs=2))
            gpool = ph2.enter_context(tc.tile_pool(name="gpool", bufs=1))
            apool = ph2.enter_context(tc.tile_pool(name="apool", bufs=2))
            opool = ph2.enter_context(tc.tile_pool(name="opool", bufs=3))
            spool = ph2.enter_context(tc.tile_pool(name="spool", bufs=4))

            SCW = 1024  # superchunk width (2 PSUM banks; one exp/accum/stt pass)

            def score_gen(h, qsb, aTs_list):
                """scores+exp+stats+combine+xbar-transpose for (h, qsb)."""
                qt_t = qt_sb[(qsb, h)]
                q1 = qt_t[0:64, :]
                q2 = qt_t[64:128, :]
                k1 = kt_sb[h][0:64, :]
                k2 = kt_sb[h][64:128, :]
                # one aTs tile per (h, qsb): 16 k-blocks x 512 t-cols
                if not aTs_list:
                    aTs_list.append(
                        apool.tile([128, 16 * CH], bf16, tag="aTs", name=f"aTs{qsb}")
                    )
                for qt in range(NQT):
                    S = qsb * TB + qt * 128 + 128
                    nsch = (S + SCW - 1) // SCW
                    l12p = spool.tile([128, 4], f32, tag="l12p", name="l12p")
                    chunks = [None] * nsch
                    # diagonal (mask-hop) superchunk first: its DVE mask + exps
                    # overlap the plain superchunks' matmuls/exps
                    for c in ([nsch - 1] + list(range(nsch - 1)) if nsch > 1 else [0]):
                        w = min(SCW, S - c * SCW)
                        ps1 = scps.tile([128, SCW], f32, tag="ps", name="ps1")
                        ps2 = scps.tile([128, SCW], f32, tag="ps", name="ps2")
                        for off in range(0, w, CH):
                            ww = min(CH, w - off)
                            nc.tensor.matmul(
                                ps1[:, off : off + ww],
                                lhsT=q1[:, qt * 128 : (qt + 1) * 128],
                                rhs=k1[:, c * SCW + off : c * SCW + off + ww],
                                start=True, stop=True,
                            )
                            nc.tensor.matmul(
                                ps2[:, off : off + ww],
                                lhsT=q2[:, qt * 128 : (qt + 1) * 128],
                                rhs=k2[:, c * SCW + off : c * SCW + off + ww],
                                start=True, stop=True,
                            )
                        if c == nsch - 1:
                            dw = w - 128
                            nc.vector.tensor_add(
                                ps1[:, dw : dw + 128], ps1[:, dw : dw + 128], msk_sb[:]
                            )
                            nc.vector.tensor_add(
                                ps2[:, dw : dw + 128], ps2[:, dw : dw + 128], msk_sb[:]
                            )
                        e1 = epool.tile([128, SCW], bf16, tag="e", name="e1")
                        e2 = epool.tile([128, SCW], bf16, tag="e", name="e2")
                        nc.scalar.activation(
                            e1[:, :w], ps1[:, :w], Act.Exp, scale=SCALE,
                            accum_out=l12p[:, c : c + 1],
                        )
                        nc.scalar.activation(
                            e2[:, :w], ps2[:, :w], Act.Exp, scale=SCALE,
                            accum_out=l12p[:, 2 + c : 3 + c],
                        )
                        chunks[c] = (e1, e2, w)
                        yield 430 * ((w + CH - 1) // CH)

                    # per-qt stats: r1 = 1/l1, ccr = -lam/l2
                    if nsch > 1:
                        l12 = spool.tile([128, 2], f32, tag="l12", name="l12")
                        nc.vector.reduce_sum(
                            l12[:], l12p[:].rearrange("p (a c) -> p a c", a=2)[:, :, :nsch],
                            axis=X,
                        )
                        s12 = l12[:]
                    else:
                        s12 = l12p[:].rearrange("p (a c) -> p a c", a=2)[:, :, 0]
                    rl12 = spool.tile([128, 2], f32, tag="rl12", name="rl12")
                    # bf16 ccr keeps the combine stt on the 2-elem/cycle path
                    ccr = spool.tile([128, 1], bf16, tag="ccr", name="ccr")
                    nc.vector.reciprocal(rl12[:], s12)
                    nc.vector.tensor_tensor(
                        ccr[:], rl12[:, 1:2], lamn_sb[:, h : h + 1], Alu.mult
                    )
                    r1 = rl12[:, 0:1]
                    dn = dpool.tile([128, 2 * SCW], bf16, tag="dn", name="dn")
                    for c, (e1, e2, w) in enumerate(chunks):
                        g = gpool.tile([128, SCW], bf16, tag="g", name="g")
                        nc.vector.tensor_scalar(g[:, :w], e1[:, :w], r1, None, Alu.mult)
                        nc.vector.scalar_tensor_tensor(
                            dn[:, c * SCW : c * SCW + w], e2[:, :w], ccr[:], g[:, :w],
                            Alu.mult, Alu.add,
                        )
                    # one xbar per q-tile: dn[q, kk*128+s'] -> aTs[s', kk*512+qt*128+q]
                    nc.sync.dma_start(
                        out=aTs_list[0][:]
                        .rearrange("p (kk r) -> p kk r", kk=16)[
                            :, : S // 128, qt * 128 : (qt + 1) * 128
                        ],
                        in_=dn[:, :S],
                        transpose=True,
                    )

            def tav_gen(h, qsb, aTs_list):
                """attn@V from xbar-transposed tiles + ot evac per column block."""
                nsc = (qsb + 1) * NQT
                av = accps.tile([128, TB], f32, tag="acc", name="av")
                for k in range(nsc):
                    j0 = 0 if k < qsb * NQT else (k - qsb * NQT)
                    nc.tensor.matmul(
                        av[:, j0 * 128 :],
                        lhsT=v_sb[k][:, h * HD : (h + 1) * HD],
                        rhs=aTs_list[0][:, k * CH + j0 * 128 : (k + 1) * CH],
                        start=(k == 0),
                        stop=(k == nsc - 1),
                    )
                    if k >= qsb * NQT:
                        tq = k - qsb * NQT
                        nc.vector.tensor_copy(
                            ot_sb[h][:, qsb * TB + tq * 128 : qsb * TB + (tq + 1) * 128],
                            av[:, tq * 128 : (tq + 1) * 128],
                        )
                    yield 220

            def q_gen(tb, h):
                xt = x_sb[tb]
                ps = prjps.tile([128, TB], f32, tag="pj", name="pj")
                for d in range(NDC):
                    nc.tensor.matmul(
                        ps[:],
                        lhsT=wslice(wq_t, d, h * HD, (h + 1) * HD),
                        rhs=xt[:, d * TB : (d + 1) * TB],
                        start=(d == 0),
                        stop=(d == NDC - 1),
                    )
                    if d % 2 == 1:
                        yield 430
                qt = qpool.tile([128, TB], bf16, tag="qt", name=f"qt{tb}_{h}")
                nc.scalar.copy(qt[:], ps[:])
                qt_sb[(tb, h)] = qt

            def k_gen(tb, h):
                xt = x_sb[tb]
                ps = prjps.tile([128, TB], f32, tag="pj", name="pj")
                for d in range(NDC):
                    nc.tensor.matmul(
                        ps[:],
                        lhsT=wslice(wk_t, d, h * HD, (h + 1) * HD),
                        rhs=xt[:, d * TB : (d + 1) * TB],
                        start=(d == 0),
                        stop=(d == NDC - 1),
                    )
                    if d % 2 == 1:
                        yield 430
                nc.vector.tensor_copy(kt_sb[h][:, tb * TB : (tb + 1) * TB], ps[:])

            def v_gen(tb, tt):
                xt = x_sb[tb]
                ps = prjps.tile([128, HL * HD], f32, tag="pj", name="pj")
                for d in range(NDC):
                    nc.tensor.matmul(
                        ps[:],
                        lhsT=xt[:, d * TB + tt * 128 : d * TB + (tt + 1) * 128],
                        rhs=wv_t[d // 4][:, (d % 4) * 512 : (d % 4 + 1) * 512],
                        start=(d == 0),
                        stop=(d == NDC - 1),
                    )
                    if d % 2 == 1:
                        yield 430
                nc.vector.tensor_copy(v_sb[tb * NQT + tt][:], ps[:])

            def op_gen(qsb, tq):
                t0 = qsb * TB + tq * 128
                for dch in range(4):
                    oev = opool.tile([128, 512], bf16, tag="oev", name="oev")
                    po = prjps.tile([128, 512], f32, tag="pj", name="po")
                    for h in range(HL):
                        nc.tensor.matmul(
                            po[:],
                            lhsT=ot_sb[h][:, t0 : t0 + 128],
                            rhs=wo_t[h // 2][
                                :, (h % 2) * D + dch * 512 : (h % 2) * D + (dch + 1) * 512
                            ],
                            start=(h == 0),
                            stop=(h == HL - 1),
                        )
                    if dch % 2 == 0:
                        nc.scalar.copy(oev[:], po[:])
                    else:
                        nc.vector.tensor_copy(oev[:], po[:])
                    nc.sync.dma_start(
                        out=out[t0 : t0 + 128, dch * 512 : (dch + 1) * 512], in_=oev[:]
                    )
                    yield 860

            def xdma_gen(tb):
                emit_xdma(tb)
                return
                yield  # pragma: no cover

            # filler generators per (qsb, head) slot
            F = {
                (0, 0): [v_gen(0, 0), v_gen(0, 1), v_gen(0, 2), v_gen(0, 3)],
                (0, 1): [xdma_gen(1), q_gen(1, 0), q_gen(1, 1)],
                (0, 2): [q_gen(1, 2), q_gen(1, 3), k_gen(1, 0)],
                (0, 3): [k_gen(1, 1), k_gen(1, 2), k_gen(1, 3)],
                (1, 0): [v_gen(1, 0), v_gen(1, 1), v_gen(1, 2), v_gen(1, 3)],
                (1, 1): [xdma_gen(2), q_gen(2, 0), q_gen(2, 1), op_gen(0, 0)],
                (1, 2): [q_gen(2, 2), q_gen(2, 3), k_gen(2, 0), op_gen(0, 1)],
                (1, 3): [k_gen(2, 1), k_gen(2, 2), k_gen(2, 3), op_gen(0, 2)],
                (2, 0): [v_gen(2, 0), v_gen(2, 1), v_gen(2, 2), v_gen(2, 3), op_gen(0, 3)],
                (2, 1): [xdma_gen(3), q_gen(3, 0), q_gen(3, 1), op_gen(1, 0)],
                (2, 2): [q_gen(3, 2), q_gen(3, 3), k_gen(3, 0), op_gen(1, 1)],
                (2, 3): [k_gen(3, 1), k_gen(3, 2), k_gen(3, 3), op_gen(1, 2)],
                (3, 0): [v_gen(3, 0), v_gen(3, 1), v_gen(3, 2), v_gen(3, 3), op_gen(1, 3)],
                (3, 1): [op_gen(2, 0), op_gen(2, 1)],
                (3, 2): [op_gen(2, 2)],
                (3, 3): [op_gen(2, 3)],
            }

            FILL_NS = 1200

            pending = None
            for qsb in range(NTB):
                for h in range(HL):
                    work = deque()
                    work.extend(F.get((qsb, h), []))
                    if pending is not None:
                        # tav FIRST: its AV reads release the previous phase's
                        # aTs tile (ring 2) early, unblocking this phase's xbars
                        work.appendleft(tav_gen(*pending))
                    aTs_list = []
                    sg = score_gen(h, qsb, aTs_list)
                    for yv in sg:
                        debt = FILL_NS * yv // 430
                        while debt > 0 and work:
                            try:
                                debt -= next(work[0])
                            except StopIteration:
                                work.popleft()
                    while work:
                        try:
                            next(work[0])
                        except StopIteration:
                            work.popleft()
                    pending = (h, qsb, aTs_list)
            tg = tav_gen(*pending)
            ops = deque(op_gen(3, tq) for tq in range(4))
            k = 0
            avail = 0
            for _ in tg:
                k += 1
                if k >= 13:
                    avail += 1
                for _ in range(2):
                    if avail and ops:
                        try:
                            next(ops[0])
                        except StopIteration:
                            ops.popleft()
            while ops:
                try:
                    next(ops[0])
                except StopIteration:
                    ops.popleft()

    nc.finalize()
    return nc


def _get_nc():
    if "nc" not in _CACHE:
        _CACHE["nc"] = _build()
    return _CACHE["nc"]


def kernel(x, Wq, Wk, Wv, Wo, lambda_init):
    from concourse.bass_utils import run_bass_kernel_spmd

    bf16 = ml_dtypes.bfloat16
    x = np.asarray(x, dtype=np.float32)
    Wq = np.asarray(Wq, dtype=np.float32)
    Wk = np.asarray(Wk, dtype=np.float32)
    Wv = np.asarray(Wv, dtype=np.float32)
    Wo = np.asarray(Wo, dtype=np.float32)
    lam_full = 1.0 / (1.0 + np.exp(-np.asarray(lambda_init, dtype=np.float32)))

    msk = np.triu(np.full((128, 128), -1e30, np.float32), k=1)  # additive causal

    # x^T rearranged per core batch: xr[p, ((tb*16)+d)*512 + j] = x[b][tb*512+j, d*128+p]
    xr_b = [
        np.ascontiguousarray(
            x[b].reshape(NTB, TB, NDC, 128).transpose(3, 0, 2, 1).reshape(128, NTB * NDC * TB)
        ).astype(bf16)
        for b in range(B)
    ]

    def wrearr(W, cols):
        # [p, d*512 + c] = W[d*128+p, cols[c]]
        Wc = W[:, cols]  # [2048, 512]
        return np.ascontiguousarray(
            Wc.reshape(NDC, 128, HL * HD).transpose(1, 0, 2).reshape(128, NDC * HL * HD)
        ).astype(bf16)

    in_maps = []
    for c in range(NCORES):
        b, g = divmod(c, NCORES // B)
        cols = slice(g * HL * HD, (g + 1) * HL * HD)
        won_r = np.ascontiguousarray(
            Wo[cols, :].reshape(HL, 128, D).transpose(1, 0, 2).reshape(128, HL * D)
        ).astype(bf16)
        in_maps.append(
            {
                "xr": xr_b[b],
                "wq": wrearr(Wq, cols),
                "wk": wrearr(Wk, cols),
                "wv": wrearr(Wv, cols),
                "won": won_r,
                "lamn": np.tile(-lam_full[g * HL : (g + 1) * HL], (128, 1)).astype(np.float32),
                "msk": msk,
            }
        )

    nc = _get_nc()
    res = run_bass_kernel_spmd(nc, in_maps, core_ids=list(range(NCORES)))
    _CACHE["last_results"] = res

    full = np.zeros((B, T, D), np.float32)
    for c in range(NCORES):
        b = c // (NCORES // B)
        full[b] += res.results[c]["out"].astype(np.float32)
    return full


# revision 37
# speedup vs baseline: 1.0317x; 1.0317x over previous
"""Differential attention (B=2, T=2048, D=2048, H=16, HD=128) on 8 TRN2 cores.

Sharding: core c -> (batch b = c // 4, head-group g = c % 4); each core runs
batch b with 4 heads (4g..4g+3). Out-projection partials are summed on host
(outputs stored bf16; host accumulates in f32).

Per-core schedule (single SPMD Bass program), changes over the 468us
baseline (measured 447-452us on the same harness; baseline re-measured
~558us in this session):
  - attn transposes moved OFF the PE: dn rows are transposed via the DMA
    XBAR (dma_start(transpose=True), 16x128 tiles; ONE instr per q-tile
    batching up to 16 128x128 blocks through a 3D out AP into a single
    [128, 8192] aTs tile per phase). PE sheds ~70k columns + 544 matmul
    overheads; the psum aT ring and its evac casts disappear. 64 XBARs
    total keeps the SP queue (which also carries x/out DMAs) unclogged.
    Removing the interleaved transposes also fixes the projection-chain
    cadence: back-to-back 512-col matmuls now issue at ~216ns (2.4GHz)
    vs 259ns in the baseline.
  - score pairs (q1k1 / q2k2, K=64 each) rely on bass's auto tile_position
    (row groups 0/64) and execute CONCURRENTLY in the PE array (verified
    4ns start stagger on HW) - scores cost N cols per pair, not 2N.
  - normalization restructured since the diag-matmul trick is gone:
    dn = e1*(1/l1) + (-lam/l2)*e2 built as g = e1*r1 (DVE tensor_scalar,
    2 elem/cycle) and dn = stt(e2, ccr, g, mult, add) on DVE with bf16
    ccr; lam uploaded NEGATED from host. No diag tiles, no idnn.
    (GpSimd was tried for g and is ~15x too slow - 6.4us per [128,512].)
  - softmax runs on 1024-wide superchunks (2-bank psum tiles, one
    exp+accum per half): halves ACTIVATE / ACTIVATION_READ_ACCUMULATOR /
    stt / XBAR instruction counts vs 512 chunks.
  - weights land in per-4d-group SBUF tiles (wq/wk/wv: 4x[128,2048], wo:
    2x[128,4096]) so first matmuls gate on 512KB not 2MB; DMA queues:
    SP=x+wv+out+xbar, Act=wq+wk+wo.
  - psum: scps 2x[128,1024] + prjps 3x[128,512] + av accumulator; the
    3-deep projection ring plus a 3-deep oev pool keeps the out-projection
    weave from stalling on out-DMA backlog.

All matmuls bf16; accumulation fp32 in PSUM, softmax statistics fp32.
"""

from contextlib import ExitStack

import ml_dtypes
import numpy as np

B, T, D = 2, 2048, 2048
H, HD = 16, 128
HHD = HD // 2
HL = 4  # heads per core
NCORES = 8
SCALE = 1.0 / float(np.sqrt(np.float32(HHD)))

TB = 512  # t-superblock (q-block rows, AV free dim)
NTB = T // TB  # 4
DC = 128  # contraction chunk (partition dim)
NDC = D // DC  # 16
NQT = TB // 128  # q-tiles (128 rows) per superblock
CH = 512  # softmax chunk width (1 PSUM bank)

_CACHE = {}


def _build():
    import concourse.mybir as mybir
    from concourse.bacc import Bacc
    from concourse.tile import TileContext

    f32 = mybir.dt.float32
    bf16 = mybir.dt.bfloat16
    Alu = mybir.AluOpType
    Act = mybir.ActivationFunctionType
    X = mybir.AxisListType.X

    nc = Bacc("TRN2", num_devices=NCORES)
    # host-rearranged inputs: xr[p, tb*8192 + d*512 + j] = x[b][tb*512+j, d*128+p]
    xr = nc.declare_dram_parameter("xr", [128, NTB * NDC * TB], bf16, isOutput=False)
    # wq/wk/wv: w_r[p, d*512 + c] = W[d*128+p, g*512 + c]
    wq = nc.declare_dram_parameter("wq", [128, NDC * HL * HD], bf16, isOutput=False)
    wk = nc.declare_dram_parameter("wk", [128, NDC * HL * HD], bf16, isOutput=False)
    wv = nc.declare_dram_parameter("wv", [128, NDC * HL * HD], bf16, isOutput=False)
    # won_r[p, h*2048 + n] = Wo[g*512 + h*128 + p, n]
    won = nc.declare_dram_parameter("won", [128, HL * D], bf16, isOutput=False)
    lamn = nc.declare_dram_parameter("lamn", [128, HL], f32, isOutput=False)  # -sigmoid
    msk = nc.declare_dram_parameter("msk", [128, 128], f32, isOutput=False)
    out = nc.declare_dram_parameter("out", [T, D], bf16, isOutput=True)

    WG = 4 * 512  # 4-d-group weight tile cols

    with TileContext(nc) as tc, ExitStack() as top:
        # ---- persistent SBUF ----
        const = top.enter_context(tc.tile_pool(name="const", bufs=1))
        lamn_sb = const.tile([128, HL], f32, tag="lam", name="lam")
        msk_sb = const.tile([128, 128], f32, tag="msk", name="msk")

        wpool = top.enter_context(tc.tile_pool(name="wpool", bufs=1))
        wq_t = [wpool.tile([128, WG], bf16, tag=f"wq{g}", name=f"wq{g}") for g in range(4)]
        wk_t = [wpool.tile([128, WG], bf16, tag=f"wk{g}", name=f"wk{g}") for g in range(4)]
        wv_t = [wpool.tile([128, WG], bf16, tag=f"wv{g}", name=f"wv{g}") for g in range(4)]
        wo_t = [wpool.tile([128, 2 * D], bf16, tag=f"wo{k}", name=f"wo{k}") for k in range(2)]

        resid = top.enter_context(tc.tile_pool(name="resid", bufs=1))
        kt_sb = [resid.tile([128, T], bf16, tag=f"kt{h}", name=f"kt{h}") for h in range(HL)]
        v_sb = [resid.tile([128, HL * HD], bf16, tag=f"v{s}", name=f"v{s}") for s in range(T // 128)]
        ot_sb = [resid.tile([128, T], bf16, tag=f"ot{h}", name=f"ot{h}") for h in range(HL)]
        # q tiles per (h, tb): alive ~1.5 superblocks -> 8-buf rotation
        qpool = top.enter_context(tc.tile_pool(name="qpool", bufs=7))
        qt_sb = {}

        xpool = top.enter_context(tc.tile_pool(name="xpool", bufs=2))
        x_sb = {}

        def wslice(tiles, d, h0c, h1c):
            """weight cols [d*512 + h0c, d*512 + h1c) from the 4-d-group tiles."""
            return tiles[d // 4][:, (d % 4) * 512 + h0c : (d % 4) * 512 + h1c]

        # ---- input DMAs ----
        def emit_xdma(tb, widths=(1024,) * 8):
            xt = xpool.tile([128, NDC * TB], bf16, tag="x", name=f"x{tb}")
            off = 0
            for cw in widths:
                nc.sync.dma_start(
                    out=xt[:, off : off + cw],
                    in_=xr[:, tb * NDC * TB + off : tb * NDC * TB + off + cw],
                )
                off += cw
            x_sb[tb] = xt

        emit_xdma(0, widths=(512, 512) + (1024,) * 7)
        for g in range(4):
            if g == 0:
                nc.scalar.dma_start(out=wq_t[0][:, 0:512], in_=wq[:, 0:512])
                nc.scalar.dma_start(out=wq_t[0][:, 512:WG], in_=wq[:, 512:WG])
            else:
                nc.scalar.dma_start(out=wq_t[g][:], in_=wq[:, g * WG : (g + 1) * WG])
        for g in range(4):
            nc.scalar.dma_start(out=wk_t[g][:], in_=wk[:, g * WG : (g + 1) * WG])
        nc.sync.dma_start(out=lamn_sb[:], in_=lamn[:])
        nc.sync.dma_start(out=msk_sb[:], in_=msk[:])
        for g in range(4):
            nc.sync.dma_start(out=wv_t[g][:], in_=wv[:, g * WG : (g + 1) * WG])
        for k in range(2):
            nc.scalar.dma_start(out=wo_t[k][:], in_=won[:, k * 2 * D : (k + 1) * 2 * D])

        # ---- prologue: Q(tb0), K(tb0) on a wide psum pool ----
        with ExitStack() as ph1:
            pps = ph1.enter_context(tc.tile_pool(name="pps", bufs=8, space="PSUM"))
            xt0 = x_sb[0]
            for w_t, store in ((wq_t, "q"), (wk_t, "k")):
                pss = [pps.tile([128, TB], f32, tag="ps", name=f"p{h}", bufs=4) for h in range(HL)]
                for d in range(NDC):
                    for h in range(HL):
                        nc.tensor.matmul(
                            pss[h][:],
                            lhsT=wslice(w_t, d, h * HD, (h + 1) * HD),
                            rhs=xt0[:, d * TB : (d + 1) * TB],
                            start=(d == 0),
                            stop=(d == NDC - 1),
                        )
                for h in range(HL):
                    if store == "q":
                        qt = qpool.tile([128, TB], bf16, tag="qt", name=f"qt0_{h}")
                        nc.vector.tensor_copy(qt[:], pss[h][:])
                        qt_sb[(0, h)] = qt
                    else:
                        nc.vector.tensor_copy(kt_sb[h][:, 0:TB], pss[h][:])

        # ---------------- attention + late projections ----------------
        from collections import deque

        with ExitStack() as ph2:
            scps = ph2.enter_context(tc.tile_pool(name="scps", bufs=2, space="PSUM"))
            prjps = ph2.enter_context(tc.tile_pool(name="prjps", bufs=3, space="PSUM"))
            accps = ph2.enter_context(tc.tile_pool(name="accps", bufs=1, space="PSUM"))
            epool = ph2.enter_context(tc.tile_pool(name="epool", bufs=5))
            dpool = ph2.enter_context(tc.tile_pool(name="dpool", bufs=2))
            gpool = ph2.enter_context(tc.tile_pool(name="gpool", bufs=1))
            apool = ph2.enter_context(tc.tile_pool(name="apool", bufs=2))
            opool = ph2.enter_context(tc.tile_pool(name="opool", bufs=3))
            spool = ph2.enter_context(tc.tile_pool(name="spool", bufs=4))

            SCW = 1024  # superchunk width (2 PSUM banks; one exp/accum/stt pass)

            def score_gen(h, qsb, aTs_list):
                """scores+exp+stats+combine+xbar-transpose for (h, qsb)."""
                qt_t = qt_sb[(qsb, h)]
                q1 = qt_t[0:64, :]
                q2 = qt_t[64:128, :]
                k1 = kt_sb[h][0:64, :]
                k2 = kt_sb[h][64:128, :]
                # one aTs tile per (h, qsb): 16 k-blocks x 512 t-cols
                if not aTs_list:
                    aTs_list.append(
                        apool.tile([128, 16 * CH], bf16, tag="aTs", name=f"aTs{qsb}")
                    )
                for qt in range(NQT):
                    S = qsb * TB + qt * 128 + 128
                    nsch = (S + SCW - 1) // SCW
                    l12p = spool.tile([128, 4], f32, tag="l12p", name="l12p")
                    chunks = [None] * nsch
                    # diagonal (mask-hop) superchunk first: its DVE mask + exps
                    # overlap the plain superchunks' matmuls/exps
                    for c in ([nsch - 1] + list(range(nsch - 1)) if nsch > 1 else [0]):
                        w = min(SCW, S - c * SCW)
                        ps1 = scps.tile([128, SCW], f32, tag="ps", name="ps1")
                        ps2 = scps.tile([128, SCW], f32, tag="ps", name="ps2")
                        for off in range(0, w, CH):
                            ww = min(CH, w - off)
                            nc.tensor.matmul(
                                ps1[:, off : off + ww],
                                lhsT=q1[:, qt * 128 : (qt + 1) * 128],
                                rhs=k1[:, c * SCW + off : c * SCW + off + ww],
                                start=True, stop=True,
                            )
                            nc.tensor.matmul(
                                ps2[:, off : off + ww],
                                lhsT=q2[:, qt * 128 : (qt + 1) * 128],
                                rhs=k2[:, c * SCW + off : c * SCW + off + ww],
                                start=True, stop=True,
                            )
                        if c == nsch - 1:
                            dw = w - 128
                            nc.vector.tensor_add(
                                ps1[:, dw : dw + 128], ps1[:, dw : dw + 128], msk_sb[:]
                            )
                            nc.vector.tensor_add(
                                ps2[:, dw : dw + 128], ps2[:, dw : dw + 128], msk_sb[:]
                            )
                        e1 = epool.tile([128, SCW], bf16, tag="e", name="e1")
                        e2 = epool.tile([128, SCW], bf16, tag="e", name="e2")
                        nc.scalar.activation(
                            e1[:, :w], ps1[:, :w], Act.Exp, scale=SCALE,
                            accum_out=l12p[:, c : c + 1],
                        )
                        nc.scalar.activation(
                            e2[:, :w], ps2[:, :w], Act.Exp, scale=SCALE,
                            accum_out=l12p[:, 2 + c : 3 + c],
                        )
                        chunks[c] = (e1, e2, w)
                        yield 430 * ((w + CH - 1) // CH)

                    # per-qt stats: r1 = 1/l1, ccr = -lam/l2
                    if nsch > 1:
                        l12 = spool.tile([128, 2], f32, tag="l12", name="l12")
                        nc.vector.reduce_sum(
                            l12[:], l12p[:].rearrange("p (a c) -> p a c", a=2)[:, :, :nsch],
                            axis=X,
                        )
                        s12 = l12[:]
                    else:
                        s12 = l12p[:].rearrange("p (a c) -> p a c", a=2)[:, :, 0]
                    rl12 = spool.tile([128, 2], f32, tag="rl12", name="rl12")
                    # bf16 ccr keeps the combine stt on the 2-elem/cycle path
                    ccr = spool.tile([128, 1], bf16, tag="ccr", name="ccr")
                    nc.vector.reciprocal(rl12[:], s12)
                    nc.vector.tensor_tensor(
                        ccr[:], rl12[:, 1:2], lamn_sb[:, h : h + 1], Alu.mult
                    )
                    r1 = rl12[:, 0:1]
                    dn = dpool.tile([128, 2 * SCW], bf16, tag="dn", name="dn")
                    for c, (e1, e2, w) in enumerate(chunks):
                        g = gpool.tile([128, SCW], bf16, tag="g", name="g")
                        nc.vector.tensor_scalar(g[:, :w], e1[:, :w], r1, None, Alu.mult)
                        nc.vector.scalar_tensor_tensor(
                            dn[:, c * SCW : c * SCW + w], e2[:, :w], ccr[:], g[:, :w],
                            Alu.mult, Alu.add,
                        )
                    # one xbar per q-tile: dn[q, kk*128+s'] -> aTs[s', kk*512+qt*128+q]
                    nc.sync.dma_start(
                        out=aTs_list[0][:]
                        .rearrange("p (kk r) -> p kk r", kk=16)[
                            :, : S // 128, qt * 128 : (qt + 1) * 128
                        ],
                        in_=dn[:, :S],
                        transpose=True,
                    )

            def tav_gen(h, qsb, aTs_list):
                """attn@V from xbar-transposed tiles + ot evac per column block."""
                nsc = (qsb + 1) * NQT
                av = accps.tile([128, TB], f32, tag="acc", name="av")
                for k in range(nsc):
                    j0 = 0 if k < qsb * NQT else (k - qsb * NQT)
                    nc.tensor.matmul(
                        av[:, j0 * 128 :],
                        lhsT=v_sb[k][:, h * HD : (h + 1) * HD],
                        rhs=aTs_list[0][:, k * CH + j0 * 128 : (k + 1) * CH],
                        start=(k == 0),
                        stop=(k == nsc - 1),
                    )
                    if k >= qsb * NQT:
                        tq = k - qsb * NQT
                        nc.vector.tensor_copy(
                            ot_sb[h][:, qsb * TB + tq * 128 : qsb * TB + (tq + 1) * 128],
                            av[:, tq * 128 : (tq + 1) * 128],
                        )
                    yield 220

            def q_gen(tb, h):
                xt = x_sb[tb]
                ps = prjps.tile([128, TB], f32, tag="pj", name="pj")
                for d in range(NDC):
                    nc.tensor.matmul(
                        ps[:],
                        lhsT=wslice(wq_t, d, h * HD, (h + 1) * HD),
                        rhs=xt[:, d * TB : (d + 1) * TB],
                        start=(d == 0),
                        stop=(d == NDC - 1),
                    )
                    if d % 2 == 1:
                        yield 430
                qt = qpool.tile([128, TB], bf16, tag="qt", name=f"qt{tb}_{h}")
                nc.scalar.copy(qt[:], ps[:])
                qt_sb[(tb, h)] = qt

            def k_gen(tb, h):
                xt = x_sb[tb]
                ps = prjps.tile([128, TB], f32, tag="pj", name="pj")
                for d in range(NDC):
                    nc.tensor.matmul(
                        ps[:],
                        lhsT=wslice(wk_t, d, h * HD, (h + 1) * HD),
                        rhs=xt[:, d * TB : (d + 1) * TB],
                        start=(d == 0),
                        stop=(d == NDC - 1),
                    )
                    if d % 2 == 1:
                        yield 430
                nc.vector.tensor_copy(kt_sb[h][:, tb * TB : (tb + 1) * TB], ps[:])

            def v_gen(tb, tt):
                xt = x_sb[tb]
                ps = prjps.tile([128, HL * HD], f32, tag="pj", name="pj")
                for d in range(NDC):
                    nc.tensor.matmul(
                        ps[:],
                        lhsT=xt[:, d * TB + tt * 128 : d * TB + (tt + 1) * 128],
                        rhs=wv_t[d // 4][:, (d % 4) * 512 : (d % 4 + 1) * 512],
                        start=(d == 0),
                        stop=(d == NDC - 1),
                    )
                    if d % 2 == 1:
                        yield 430
                nc.vector.tensor_copy(v_sb[tb * NQT + tt][:], ps[:])

            def op_gen(qsb, tq):
                t0 = qsb * TB + tq * 128
                for dch in range(4):
                    oev = opool.tile([128, 512], bf16, tag="oev", name="oev")
                    po = prjps.tile([128, 512], f32, tag="pj", name="po")
                    for h in range(HL):
                        nc.tensor.matmul(
                            po[:],
                            lhsT=ot_sb[h][:, t0 : t0 + 128],
                            rhs=wo_t[h // 2][
                                :, (h % 2) * D + dch * 512 : (h % 2) * D + (dch + 1) * 512
                            ],
                            start=(h == 0),
                            stop=(h == HL - 1),
                        )
                    if dch % 2 == 0:
                        nc.scalar.copy(oev[:], po[:])
                    else:
                        nc.vector.tensor_copy(oev[:], po[:])
                    nc.sync.dma_start(
                        out=out[t0 : t0 + 128, dch * 512 : (dch + 1) * 512], in_=oev[:]
                    )
                    yield 860

            def xdma_gen(tb):
                emit_xdma(tb)
                return
                yield  # pragma: no cover

            # filler generators per (qsb, head) slot
            F = {
                (0, 0): [v_gen(0, 0), v_gen(0, 1), v_gen(0, 2), v_gen(0, 3)],
                (0, 1): [xdma_gen(1), q_gen(1, 0), q_gen(1, 1)],
                (0, 2): [q_gen(1, 2), q_gen(1, 3), k_gen(1, 0)],
                (0, 3): [k_gen(1, 1), k_gen(1, 2), k_gen(1, 3)],
                (1, 0): [v_gen(1, 0), v_gen(1, 1), v_gen(1, 2), v_gen(1, 3)],
                (1, 1): [xdma_gen(2), q_gen(2, 0), q_gen(2, 1), op_gen(0, 0)],
                (1, 2): [q_gen(2, 2), q_gen(2, 3), k_gen(2, 0), op_gen(0, 1)],
                (1, 3): [k_gen(2, 1), k_gen(2, 2), k_gen(2, 3), op_gen(0, 2)],
                (2, 0): [v_gen(2, 0), v_gen(2, 1), v_gen(2, 2), v_gen(2, 3), op_gen(0, 3)],
                (2, 1): [xdma_gen(3), q_gen(3, 0), q_gen(3, 1), op_gen(1, 0)],
                (2, 2): [q_gen(3, 2), q_gen(3, 3), k_gen(3, 0), op_gen(1, 1)],
                (2, 3): [k_gen(3, 1), k_gen(3, 2), k_gen(3, 3), op_gen(1, 2)],
                (3, 0): [v_gen(3, 0), v_gen(3, 1), v_gen(3, 2), v_gen(3, 3), op_gen(1, 3)],
                (3, 1): [op_gen(2, 0), op_gen(2, 1)],
                (3, 2): [op_gen(2, 2)],
                (3, 3): [op_gen(2, 3)],
            }

            FILL_NS = 1200

            pending = None
            for qsb in range(NTB):
                for h in range(HL):
                    work = deque()
                    work.extend(F.get((qsb, h), []))
                    if pending is not None:
                        work.append(tav_gen(*pending))
                    aTs_list = []
                    sg = score_gen(h, qsb, aTs_list)
                    for yv in sg:
                        debt = FILL_NS * yv // 430
                        while debt > 0 and work:
                            try:
                                debt -= next(work[0])
                            except StopIteration:
                                work.popleft()
                    while work:
                        try:
                            next(work[0])
                        except StopIteration:
                            work.popleft()
                    pending = (h, qsb, aTs_list)
            tg = tav_gen(*pending)
            ops = deque(op_gen(3, tq) for tq in range(4))
            k = 0
            avail = 0
            for _ in tg:
                k += 1
                if k >= 13:
                    avail += 1
                for _ in range(2):
                    if avail and ops:
                        try:
                            next(ops[0])
                        except StopIteration:
                            ops.popleft()
            while ops:
                try:
                    next(ops[0])
                except StopIteration:
                    ops.popleft()

    nc.finalize()
    return nc


def _get_nc():
    if "nc" not in _CACHE:
        _CACHE["nc"] = _build()
    return _CACHE["nc"]


def kernel(x, Wq, Wk, Wv, Wo, lambda_init):
    from concourse.bass_utils import run_bass_kernel_spmd

    bf16 = ml_dtypes.bfloat16
    x = np.asarray(x, dtype=np.float32)
    Wq = np.asarray(Wq, dtype=np.float32)
    Wk = np.asarray(Wk, dtype=np.float32)
    Wv = np.asarray(Wv, dtype=np.float32)
    Wo = np.asarray(Wo, dtype=np.float32)
    lam_full = 1.0 / (1.0 + np.exp(-np.asarray(lambda_init, dtype=np.float32)))

    msk = np.triu(np.full((128, 128), -1e30, np.float32), k=1)  # additive causal

    # x^T rearranged per core batch: xr[p, ((tb*16)+d)*512 + j] = x[b][tb*512+j, d*128+p]
    xr_b = [
        np.ascontiguousarray(
            x[b].reshape(NTB, TB, NDC, 128).transpose(3, 0, 2, 1).reshape(128, NTB * NDC * TB)
        ).astype(bf16)
        for b in range(B)
    ]

    def wrearr(W, cols):
        # [p, d*512 + c] = W[d*128+p, cols[c]]
        Wc = W[:, cols]  # [2048, 512]
        return np.ascontiguousarray(
            Wc.reshape(NDC, 128, HL * HD).transpose(1, 0, 2).reshape(128, NDC * HL * HD)
        ).astype(bf16)

    in_maps = []
    for c in range(NCORES):
        b, g = divmod(c, NCORES // B)
        cols = slice(g * HL * HD, (g + 1) * HL * HD)
        won_r = np.ascontiguousarray(
            Wo[cols, :].reshape(HL, 128, D).transpose(1, 0, 2).reshape(128, HL * D)
        ).astype(bf16)
        in_maps.append(
            {
                "xr": xr_b[b],
                "wq": wrearr(Wq, cols),
                "wk": wrearr(Wk, cols),
                "wv": wrearr(Wv, cols),
                "won": won_r,
                "lamn": np.tile(-lam_full[g * HL : (g + 1) * HL], (128, 1)).astype(np.float32),
                "msk": msk,
            }
        )

    nc = _get_nc()
    res = run_bass_kernel_spmd(nc, in_maps, core_ids=list(range(NCORES)))
    _CACHE["last_results"] = res

    full = np.zeros((B, T, D), np.float32)
    for c in range(NCORES):
        b = c // (NCORES // B)
        full[b] += res.results[c]["out"].astype(np.float32)
    return full


# revision 38
# speedup vs baseline: 1.0318x; 1.0001x over previous
"""Differential attention (B=2, T=2048, D=2048, H=16, HD=128) on 8 TRN2 cores.

Sharding: core c -> (batch b = c // 4, head-group g = c % 4); each core runs
batch b with 4 heads (4g..4g+3). Out-projection partials are summed on host
(outputs stored bf16; host accumulates in f32).

Per-core schedule (single SPMD Bass program), changes over the 468us
baseline (measured 447-452us on the same harness; baseline re-measured
~558us in this session):
  - attn transposes moved OFF the PE: dn rows are transposed via the DMA
    XBAR (dma_start(transpose=True), 16x128 tiles; ONE instr per q-tile
    batching up to 16 128x128 blocks through a 3D out AP into a single
    [128, 8192] aTs tile per phase). PE sheds ~70k columns + 544 matmul
    overheads; the psum aT ring and its evac casts disappear. 64 XBARs
    total keeps the SP queue (which also carries x/out DMAs) unclogged.
    Removing the interleaved transposes also fixes the projection-chain
    cadence: back-to-back 512-col matmuls now issue at ~216ns (2.4GHz)
    vs 259ns in the baseline.
  - score pairs (q1k1 / q2k2, K=64 each) rely on bass's auto tile_position
    (row groups 0/64) and execute CONCURRENTLY in the PE array (verified
    4ns start stagger on HW) - scores cost N cols per pair, not 2N.
  - normalization restructured since the diag-matmul trick is gone:
    dn = e1*(1/l1) + (-lam/l2)*e2 built as g = e1*r1 (DVE tensor_scalar,
    2 elem/cycle) and dn = stt(e2, ccr, g, mult, add) on DVE with bf16
    ccr; lam uploaded NEGATED from host. No diag tiles, no idnn.
    (GpSimd was tried for g and is ~15x too slow - 6.4us per [128,512].)
  - softmax runs on 1024-wide superchunks (2-bank psum tiles, one
    exp+accum per half): halves ACTIVATE / ACTIVATION_READ_ACCUMULATOR /
    stt / XBAR instruction counts vs 512 chunks.
  - weights land in per-4d-group SBUF tiles (wq/wk/wv: 4x[128,2048], wo:
    2x[128,4096]) so first matmuls gate on 512KB not 2MB; DMA queues:
    SP=x+wv+out+xbar, Act=wq+wk+wo.
  - psum: scps 2x[128,1024] + prjps 3x[128,512] + av accumulator; the
    3-deep projection ring plus a 3-deep oev pool keeps the out-projection
    weave from stalling on out-DMA backlog.

All matmuls bf16; accumulation fp32 in PSUM, softmax statistics fp32.
"""

from contextlib import ExitStack

import ml_dtypes
import numpy as np

B, T, D = 2, 2048, 2048
H, HD = 16, 128
HHD = HD // 2
HL = 4  # heads per core
NCORES = 8
SCALE = 1.0 / float(np.sqrt(np.float32(HHD)))

TB = 512  # t-superblock (q-block rows, AV free dim)
NTB = T // TB  # 4
DC = 128  # contraction chunk (partition dim)
NDC = D // DC  # 16
NQT = TB // 128  # q-tiles (128 rows) per superblock
CH = 512  # softmax chunk width (1 PSUM bank)

_CACHE = {}


def _build():
    import concourse.mybir as mybir
    from concourse.bacc import Bacc
    from concourse.tile import TileContext

    f32 = mybir.dt.float32
    bf16 = mybir.dt.bfloat16
    Alu = mybir.AluOpType
    Act = mybir.ActivationFunctionType
    X = mybir.AxisListType.X

    nc = Bacc("TRN2", num_devices=NCORES)
    # host-rearranged inputs: xr[p, tb*8192 + d*512 + j] = x[b][tb*512+j, d*128+p]
    xr = nc.declare_dram_parameter("xr", [128, NTB * NDC * TB], bf16, isOutput=False)
    # wq/wk/wv: w_r[p, d*512 + c] = W[d*128+p, g*512 + c]
    wq = nc.declare_dram_parameter("wq", [128, NDC * HL * HD], bf16, isOutput=False)
    wk = nc.declare_dram_parameter("wk", [128, NDC * HL * HD], bf16, isOutput=False)
    wv = nc.declare_dram_parameter("wv", [128, NDC * HL * HD], bf16, isOutput=False)
    # won_r[p, h*2048 + n] = Wo[g*512 + h*128 + p, n]
    won = nc.declare_dram_parameter("won", [128, HL * D], bf16, isOutput=False)
    lamn = nc.declare_dram_parameter("lamn", [128, HL], f32, isOutput=False)  # -sigmoid
    msk = nc.declare_dram_parameter("msk", [128, 128], f32, isOutput=False)
    out = nc.declare_dram_parameter("out", [T, D], bf16, isOutput=True)

    WG = 4 * 512  # 4-d-group weight tile cols

    with TileContext(nc) as tc, ExitStack() as top:
        # ---- persistent SBUF ----
        const = top.enter_context(tc.tile_pool(name="const", bufs=1))
        lamn_sb = const.tile([128, HL], f32, tag="lam", name="lam")
        msk_sb = const.tile([128, 128], f32, tag="msk", name="msk")

        wpool = top.enter_context(tc.tile_pool(name="wpool", bufs=1))
        wq_t = [wpool.tile([128, WG], bf16, tag=f"wq{g}", name=f"wq{g}") for g in range(4)]
        wk_t = [wpool.tile([128, WG], bf16, tag=f"wk{g}", name=f"wk{g}") for g in range(4)]
        wv_t = [wpool.tile([128, WG], bf16, tag=f"wv{g}", name=f"wv{g}") for g in range(4)]
        wo_t = [wpool.tile([128, 2 * D], bf16, tag=f"wo{k}", name=f"wo{k}") for k in range(2)]

        resid = top.enter_context(tc.tile_pool(name="resid", bufs=1))
        kt_sb = [resid.tile([128, T], bf16, tag=f"kt{h}", name=f"kt{h}") for h in range(HL)]
        v_sb = [resid.tile([128, HL * HD], bf16, tag=f"v{s}", name=f"v{s}") for s in range(T // 128)]
        ot_sb = [resid.tile([128, T], bf16, tag=f"ot{h}", name=f"ot{h}") for h in range(HL)]
        # q tiles per (h, tb): alive ~1.5 superblocks -> 8-buf rotation
        qpool = top.enter_context(tc.tile_pool(name="qpool", bufs=7))
        qt_sb = {}

        xpool = top.enter_context(tc.tile_pool(name="xpool", bufs=2))
        x_sb = {}

        def wslice(tiles, d, h0c, h1c):
            """weight cols [d*512 + h0c, d*512 + h1c) from the 4-d-group tiles."""
            return tiles[d // 4][:, (d % 4) * 512 + h0c : (d % 4) * 512 + h1c]

        # ---- input DMAs ----
        def emit_xdma(tb, widths=(1024,) * 8):
            xt = xpool.tile([128, NDC * TB], bf16, tag="x", name=f"x{tb}")
            off = 0
            for cw in widths:
                nc.sync.dma_start(
                    out=xt[:, off : off + cw],
                    in_=xr[:, tb * NDC * TB + off : tb * NDC * TB + off + cw],
                )
                off += cw
            x_sb[tb] = xt

        emit_xdma(0, widths=(512, 512) + (1024,) * 7)
        for g in range(4):
            if g == 0:
                nc.scalar.dma_start(out=wq_t[0][:, 0:512], in_=wq[:, 0:512])
                nc.scalar.dma_start(out=wq_t[0][:, 512:WG], in_=wq[:, 512:WG])
            else:
                nc.scalar.dma_start(out=wq_t[g][:], in_=wq[:, g * WG : (g + 1) * WG])
        for g in range(4):
            nc.scalar.dma_start(out=wk_t[g][:], in_=wk[:, g * WG : (g + 1) * WG])
        nc.sync.dma_start(out=lamn_sb[:], in_=lamn[:])
        nc.sync.dma_start(out=msk_sb[:], in_=msk[:])
        for g in range(4):
            nc.sync.dma_start(out=wv_t[g][:], in_=wv[:, g * WG : (g + 1) * WG])
        for k in range(2):
            nc.scalar.dma_start(out=wo_t[k][:], in_=won[:, k * 2 * D : (k + 1) * 2 * D])

        # ---- prologue: Q(tb0), K(tb0) on a wide psum pool ----
        with ExitStack() as ph1:
            pps = ph1.enter_context(tc.tile_pool(name="pps", bufs=8, space="PSUM"))
            xt0 = x_sb[0]
            for w_t, store in ((wq_t, "q"), (wk_t, "k")):
                pss = [pps.tile([128, TB], f32, tag="ps", name=f"p{h}", bufs=4) for h in range(HL)]
                for d in range(NDC):
                    for h in range(HL):
                        nc.tensor.matmul(
                            pss[h][:],
                            lhsT=wslice(w_t, d, h * HD, (h + 1) * HD),
                            rhs=xt0[:, d * TB : (d + 1) * TB],
                            start=(d == 0),
                            stop=(d == NDC - 1),
                        )
                for h in range(HL):
                    if store == "q":
                        qt = qpool.tile([128, TB], bf16, tag="qt", name=f"qt0_{h}")
                        nc.vector.tensor_copy(qt[:], pss[h][:])
                        qt_sb[(0, h)] = qt
                    else:
                        nc.vector.tensor_copy(kt_sb[h][:, 0:TB], pss[h][:])

        # ---------------- attention + late projections ----------------
        from collections import deque

        with ExitStack() as ph2:
            scps = ph2.enter_context(tc.tile_pool(name="scps", bufs=2, space="PSUM"))
            prjps = ph2.enter_context(tc.tile_pool(name="prjps", bufs=3, space="PSUM"))
            accps = ph2.enter_context(tc.tile_pool(name="accps", bufs=1, space="PSUM"))
            epool = ph2.enter_context(tc.tile_pool(name="epool", bufs=5))
            dpool = ph2.enter_context(tc.tile_pool(name="dpool", bufs=2))
            gpool = ph2.enter_context(tc.tile_pool(name="gpool", bufs=1))
            apool = ph2.enter_context(tc.tile_pool(name="apool", bufs=2))
            opool = ph2.enter_context(tc.tile_pool(name="opool", bufs=3))
            spool = ph2.enter_context(tc.tile_pool(name="spool", bufs=4))

            SCW = 1024  # superchunk width (2 PSUM banks; one exp/accum/stt pass)

            def score_gen(h, qsb, aTs_list):
                """scores+exp+stats+combine+xbar-transpose for (h, qsb)."""
                qt_t = qt_sb[(qsb, h)]
                q1 = qt_t[0:64, :]
                q2 = qt_t[64:128, :]
                k1 = kt_sb[h][0:64, :]
                k2 = kt_sb[h][64:128, :]
                # one aTs tile per (h, qsb): 16 k-blocks x 512 t-cols
                if not aTs_list:
                    aTs_list.append(
                        apool.tile([128, 16 * CH], bf16, tag="aTs", name=f"aTs{qsb}")
                    )
                for qt in range(NQT):
                    S = qsb * TB + qt * 128 + 128
                    nsch = (S + SCW - 1) // SCW
                    l12p = spool.tile([128, 4], f32, tag="l12p", name="l12p")
                    chunks = [None] * nsch
                    # diagonal (mask-hop) superchunk first: its DVE mask + exps
                    # overlap the plain superchunks' matmuls/exps
                    for c in ([nsch - 1] + list(range(nsch - 1)) if nsch > 1 else [0]):
                        w = min(SCW, S - c * SCW)
                        ps1 = scps.tile([128, SCW], f32, tag="ps", name="ps1")
                        ps2 = scps.tile([128, SCW], f32, tag="ps", name="ps2")
                        for off in range(0, w, CH):
                            ww = min(CH, w - off)
                            nc.tensor.matmul(
                                ps1[:, off : off + ww],
                                lhsT=q1[:, qt * 128 : (qt + 1) * 128],
                                rhs=k1[:, c * SCW + off : c * SCW + off + ww],
                                start=True, stop=True,
                            )
                            nc.tensor.matmul(
                                ps2[:, off : off + ww],
                                lhsT=q2[:, qt * 128 : (qt + 1) * 128],
                                rhs=k2[:, c * SCW + off : c * SCW + off + ww],
                                start=True, stop=True,
                            )
                        if c == nsch - 1:
                            dw = w - 128
                            nc.vector.tensor_add(
                                ps1[:, dw : dw + 128], ps1[:, dw : dw + 128], msk_sb[:]
                            )
                            nc.vector.tensor_add(
                                ps2[:, dw : dw + 128], ps2[:, dw : dw + 128], msk_sb[:]
                            )
                        e1 = epool.tile([128, SCW], bf16, tag="e", name="e1")
                        e2 = epool.tile([128, SCW], bf16, tag="e", name="e2")
                        nc.scalar.activation(
                            e1[:, :w], ps1[:, :w], Act.Exp, scale=SCALE,
                            accum_out=l12p[:, c : c + 1],
                        )
                        nc.scalar.activation(
                            e2[:, :w], ps2[:, :w], Act.Exp, scale=SCALE,
                            accum_out=l12p[:, 2 + c : 3 + c],
                        )
                        chunks[c] = (e1, e2, w)
                        yield 430 * ((w + CH - 1) // CH)

                    # per-qt stats: r1 = 1/l1, ccr = -lam/l2
                    if nsch > 1:
                        l12 = spool.tile([128, 2], f32, tag="l12", name="l12")
                        nc.vector.reduce_sum(
                            l12[:], l12p[:].rearrange("p (a c) -> p a c", a=2)[:, :, :nsch],
                            axis=X,
                        )
                        s12 = l12[:]
                    else:
                        s12 = l12p[:].rearrange("p (a c) -> p a c", a=2)[:, :, 0]
                    rl12 = spool.tile([128, 2], f32, tag="rl12", name="rl12")
                    # bf16 ccr keeps the combine stt on the 2-elem/cycle path
                    ccr = spool.tile([128, 1], bf16, tag="ccr", name="ccr")
                    nc.vector.reciprocal(rl12[:], s12)
                    nc.vector.tensor_tensor(
                        ccr[:], rl12[:, 1:2], lamn_sb[:, h : h + 1], Alu.mult
                    )
                    r1 = rl12[:, 0:1]
                    dn = dpool.tile([128, 2 * SCW], bf16, tag="dn", name="dn")
                    for c, (e1, e2, w) in enumerate(chunks):
                        g = gpool.tile([128, SCW], bf16, tag="g", name="g")
                        nc.vector.tensor_scalar(g[:, :w], e1[:, :w], r1, None, Alu.mult)
                        nc.vector.scalar_tensor_tensor(
                            dn[:, c * SCW : c * SCW + w], e2[:, :w], ccr[:], g[:, :w],
                            Alu.mult, Alu.add,
                        )
                    # one xbar per q-tile: dn[q, kk*128+s'] -> aTs[s', kk*512+qt*128+q]
                    nc.sync.dma_start(
                        out=aTs_list[0][:]
                        .rearrange("p (kk r) -> p kk r", kk=16)[
                            :, : S // 128, qt * 128 : (qt + 1) * 128
                        ],
                        in_=dn[:, :S],
                        transpose=True,
                    )

            def tav_gen(h, qsb, aTs_list):
                """attn@V from xbar-transposed tiles + ot evac per column block."""
                nsc = (qsb + 1) * NQT
                av = accps.tile([128, TB], f32, tag="acc", name="av")
                for k in range(nsc):
                    j0 = 0 if k < qsb * NQT else (k - qsb * NQT)
                    nc.tensor.matmul(
                        av[:, j0 * 128 :],
                        lhsT=v_sb[k][:, h * HD : (h + 1) * HD],
                        rhs=aTs_list[0][:, k * CH + j0 * 128 : (k + 1) * CH],
                        start=(k == 0),
                        stop=(k == nsc - 1),
                    )
                    if k >= qsb * NQT:
                        tq = k - qsb * NQT
                        nc.vector.tensor_copy(
                            ot_sb[h][:, qsb * TB + tq * 128 : qsb * TB + (tq + 1) * 128],
                            av[:, tq * 128 : (tq + 1) * 128],
                        )
                    yield 220

            def q_gen(tb, h):
                xt = x_sb[tb]
                ps = prjps.tile([128, TB], f32, tag="pj", name="pj")
                for d in range(NDC):
                    nc.tensor.matmul(
                        ps[:],
                        lhsT=wslice(wq_t, d, h * HD, (h + 1) * HD),
                        rhs=xt[:, d * TB : (d + 1) * TB],
                        start=(d == 0),
                        stop=(d == NDC - 1),
                    )
                    if d % 2 == 1:
                        yield 430
                qt = qpool.tile([128, TB], bf16, tag="qt", name=f"qt{tb}_{h}")
                nc.scalar.copy(qt[:], ps[:])
                qt_sb[(tb, h)] = qt

            def k_gen(tb, h):
                xt = x_sb[tb]
                ps = prjps.tile([128, TB], f32, tag="pj", name="pj")
                for d in range(NDC):
                    nc.tensor.matmul(
                        ps[:],
                        lhsT=wslice(wk_t, d, h * HD, (h + 1) * HD),
                        rhs=xt[:, d * TB : (d + 1) * TB],
                        start=(d == 0),
                        stop=(d == NDC - 1),
                    )
                    if d % 2 == 1:
                        yield 430
                nc.vector.tensor_copy(kt_sb[h][:, tb * TB : (tb + 1) * TB], ps[:])

            def v_gen(tb, tt):
                xt = x_sb[tb]
                ps = prjps.tile([128, HL * HD], f32, tag="pj", name="pj")
                for d in range(NDC):
                    nc.tensor.matmul(
                        ps[:],
                        lhsT=xt[:, d * TB + tt * 128 : d * TB + (tt + 1) * 128],
                        rhs=wv_t[d // 4][:, (d % 4) * 512 : (d % 4 + 1) * 512],
                        start=(d == 0),
                        stop=(d == NDC - 1),
                    )
                    if d % 2 == 1:
                        yield 430
                nc.vector.tensor_copy(v_sb[tb * NQT + tt][:], ps[:])

            def op_gen(qsb, tq):
                t0 = qsb * TB + tq * 128
                for dch in range(4):
                    oev = opool.tile([128, 512], bf16, tag="oev", name="oev")
                    po = prjps.tile([128, 512], f32, tag="pj", name="po")
                    for h in range(HL):
                        nc.tensor.matmul(
                            po[:],
                            lhsT=ot_sb[h][:, t0 : t0 + 128],
                            rhs=wo_t[h // 2][
                                :, (h % 2) * D + dch * 512 : (h % 2) * D + (dch + 1) * 512
                            ],
                            start=(h == 0),
                            stop=(h == HL - 1),
                        )
                    if dch % 2 == 0:
                        nc.scalar.copy(oev[:], po[:])
                    else:
                        nc.vector.tensor_copy(oev[:], po[:])
                    nc.sync.dma_start(
                        out=out[t0 : t0 + 128, dch * 512 : (dch + 1) * 512], in_=oev[:]
                    )
                    yield 860

            def xdma_gen(tb):
                emit_xdma(tb)
                return
                yield  # pragma: no cover

            # filler generators per (qsb, head) slot
            F = {
                (0, 0): [v_gen(0, 0), v_gen(0, 1), v_gen(0, 2), v_gen(0, 3)],
                (0, 1): [xdma_gen(1), q_gen(1, 0), q_gen(1, 1)],
                (0, 2): [q_gen(1, 2), q_gen(1, 3), k_gen(1, 0)],
                (0, 3): [k_gen(1, 1), k_gen(1, 2), k_gen(1, 3)],
                (1, 0): [v_gen(1, 0), v_gen(1, 1), v_gen(1, 2), v_gen(1, 3)],
                (1, 1): [xdma_gen(2), q_gen(2, 0), q_gen(2, 1), op_gen(0, 0)],
                (1, 2): [q_gen(2, 2), q_gen(2, 3), k_gen(2, 0), op_gen(0, 1)],
                (1, 3): [k_gen(2, 1), k_gen(2, 2), k_gen(2, 3), op_gen(0, 2)],
                (2, 0): [v_gen(2, 0), v_gen(2, 1), v_gen(2, 2), v_gen(2, 3), op_gen(0, 3)],
                (2, 1): [xdma_gen(3), q_gen(3, 0), q_gen(3, 1), op_gen(1, 0)],
                (2, 2): [q_gen(3, 2), q_gen(3, 3), k_gen(3, 0), op_gen(1, 1)],
                (2, 3): [k_gen(3, 1), k_gen(3, 2), k_gen(3, 3), op_gen(1, 2)],
                (3, 0): [v_gen(3, 0), v_gen(3, 1), v_gen(3, 2), v_gen(3, 3), op_gen(1, 3)],
                (3, 1): [op_gen(2, 0), op_gen(2, 1)],
                (3, 2): [op_gen(2, 2)],
                (3, 3): [op_gen(2, 3)],
            }

            FILL_NS = 1200

            pending = None
            for qsb in range(NTB):
                for h in range(HL):
                    work = deque()
                    work.extend(F.get((qsb, h), []))
                    if pending is not None:
                        # tav after the first filler: late enough that the
                        # previous phase's last xbars have landed, early
                        # enough to release its aTs tile (ring 2) promptly
                        work.insert(min(1, len(work)), tav_gen(*pending))
                    aTs_list = []
                    sg = score_gen(h, qsb, aTs_list)
                    for yv in sg:
                        debt = FILL_NS * yv // 430
                        while debt > 0 and work:
                            try:
                                debt -= next(work[0])
                            except StopIteration:
                                work.popleft()
                    while work:
                        try:
                            next(work[0])
                        except StopIteration:
                            work.popleft()
                    pending = (h, qsb, aTs_list)
            tg = tav_gen(*pending)
            ops = deque(op_gen(3, tq) for tq in range(4))
            k = 0
            avail = 0
            for _ in tg:
                k += 1
                if k >= 13:
                    avail += 1
                for _ in range(2):
                    if avail and ops:
                        try:
                            next(ops[0])
                        except StopIteration:
                            ops.popleft()
            while ops:
                try:
                    next(ops[0])
                except StopIteration:
                    ops.popleft()

    nc.finalize()
    return nc


def _get_nc():
    if "nc" not in _CACHE:
        _CACHE["nc"] = _build()
    return _CACHE["nc"]


def kernel(x, Wq, Wk, Wv, Wo, lambda_init):
    from concourse.bass_utils import run_bass_kernel_spmd

    bf16 = ml_dtypes.bfloat16
    x = np.asarray(x, dtype=np.float32)
    Wq = np.asarray(Wq, dtype=np.float32)
    Wk = np.asarray(Wk, dtype=np.float32)
    Wv = np.asarray(Wv, dtype=np.float32)
    Wo = np.asarray(Wo, dtype=np.float32)
    lam_full = 1.0 / (1.0 + np.exp(-np.asarray(lambda_init, dtype=np.float32)))

    msk = np.triu(np.full((128, 128), -1e30, np.float32), k=1)  # additive causal

    # x^T rearranged per core batch: xr[p, ((tb*16)+d)*512 + j] = x[b][tb*512+j, d*128+p]
    xr_b = [
        np.ascontiguousarray(
            x[b].reshape(NTB, TB, NDC, 128).transpose(3, 0, 2, 1).reshape(128, NTB * NDC * TB)
        ).astype(bf16)
        for b in range(B)
    ]

    def wrearr(W, cols):
        # [p, d*512 + c] = W[d*128+p, cols[c]]
        Wc = W[:, cols]  # [2048, 512]
        return np.ascontiguousarray(
            Wc.reshape(NDC, 128, HL * HD).transpose(1, 0, 2).reshape(128, NDC * HL * HD)
        ).astype(bf16)

    in_maps = []
    for c in range(NCORES):
        b, g = divmod(c, NCORES // B)
        cols = slice(g * HL * HD, (g + 1) * HL * HD)
        won_r = np.ascontiguousarray(
            Wo[cols, :].reshape(HL, 128, D).transpose(1, 0, 2).reshape(128, HL * D)
        ).astype(bf16)
        in_maps.append(
            {
                "xr": xr_b[b],
                "wq": wrearr(Wq, cols),
                "wk": wrearr(Wk, cols),
                "wv": wrearr(Wv, cols),
                "won": won_r,
                "lamn": np.tile(-lam_full[g * HL : (g + 1) * HL], (128, 1)).astype(np.float32),
                "msk": msk,
            }
        )

    nc = _get_nc()
    res = run_bass_kernel_spmd(nc, in_maps, core_ids=list(range(NCORES)))
    _CACHE["last_results"] = res

    full = np.zeros((B, T, D), np.float32)
    for c in range(NCORES):
        b = c // (NCORES // B)
        full[b] += res.results[c]["out"].astype(np.float32)
    return full
